# revision 1
# baseline (speedup 1.0000x reference)
"""Trainium2 Bass kernel for nn_ContrastiveDistortion (symmetric pairwise-KL InfoNCE loss).

Math: with IS_SYMMETRIC=True the logdet terms cancel. Let p = 1/sigma^2,
q = mu^2 + sigma^2, m2 = -2*mu*p, pq = p*q. Then (up to per-row constants that
cancel in log-softmax and a uniform +D shift)
  U'[a,b] = p_a.q_b + q_a.p_b + m2_a.mu_b + mu_a.m2_b + colsum(pq)[b]
and logits = -SCL*U' with SCL = 1/(4*T). Five K=128 matmul chunks per output
tile (vs 7 in the naive trace/quad split). Each of the 8 cores gets the full
[128,4096] feature-major mu/sigma in bf16, column-ROTATED by 512*k so the
program is SPMD-identical: the core's own 512-row block is local columns
0..511 (diagonal masked there via an extra (1e30*I, onehot) matmul chunk) and
the positive pairs are local columns 2048..2559.

The [128,1024]-wide PSUM tiles give 4 in-flight accumulation slots, so the PE
streams matmuls without waiting on consumers (row-min on DVE straight from
PSUM, exp+row-sum on Act, positive extraction on DVE). Walrus allows one sync
wait per matmul, so each matmul's (lhsT, rhs) pair is written by a single
engine: h0 planes by Act(squares)+DVE, h1 rhs slabs s2/s3 by the otherwise
idle Pool engine with Pool-copied [128,512] lhsT mirrors. The per-row
logsumexp tail over the per-tile (mrow, esum) pairs runs on host in float64.
"""

import sys
from contextlib import ExitStack

import numpy as np

sys.path.insert(0, "/opt/trn_rl_repo")

import concourse.bass as bass
import concourse.bacc as bacc_mod
import concourse.mybir as mybir
from concourse.bass_utils import run_bass_kernel_spmd
from concourse.tile import TileContext

F32 = mybir.dt.float32
BF16 = mybir.dt.bfloat16
I32 = mybir.dt.int32
AF = mybir.ActivationFunctionType
ALU = mybir.AluOpType
AX = mybir.AxisListType

P = 128          # partitions / feature dim D
NB = 4096        # N = 2B rows
NC = 8           # cores
RB = NB // NC    # 512 rows per core
NM = RB // P     # 4 m-chunks of 128 rows
HALF = NB // 2   # 2048 columns per softmax half
TW = 1024        # PSUM tile width (2 banks -> 4 slots)
NT = NB // TW    # 4 column tiles per m-chunk row block
TEMPERATURE = 0.1
WEIGHT = 5.0
SCL = 1.0 / (4.0 * TEMPERATURE)  # 2.5: l = -SCL*U' + const_row
BIG = 1e30

# DMA/prep slab splits within each 2048-col half (small first slabs so the
# first matmul group can start early).
SLABS = [(0, 512), (512, 1024), (1024, 2048)]


def _build_nc():
    nc = bacc_mod.Bacc(None, target_bir_lowering=False, name="contrastive_distortion")
    muT_d = nc.declare_dram_parameter("muT", [P, NB], BF16, isOutput=False)
    sgT_d = nc.declare_dram_parameter("sigmaT", [P, NB], BF16, isOutput=False)
    # out columns: 0:17 = per-tile row-min of U'; 17:34 = per-tile esum;
    # 34:38 = positive U' per m. Tiles are indexed h*8+m*2+th, with the last
    # tile split into two 512-col pairs (indices 15, 16).
    out_d = nc.declare_dram_parameter("out", [P, 38], F32, isOutput=True)

    with TileContext(nc) as tc, ExitStack() as ctx:
        big = ctx.enter_context(tc.tile_pool(name="big", bufs=1))
        sm = ctx.enter_context(tc.tile_pool(name="sm", bufs=1))
        scr = ctx.enter_context(tc.tile_pool(name="scr", bufs=2))
        pp = ctx.enter_context(tc.tile_pool(name="pp", bufs=4, space="PSUM"))

        # persistent planes, feature-major [128, 4096] bf16
        mu = big.tile([P, NB], BF16)    # DMA-written
        sg = big.tile([P, NB], BF16)    # DMA-written
        var = big.tile([P, NB], BF16)   # Act: sg^2
        msq = big.tile([P, NB], BF16)   # Act: mu^2
        p_ = big.tile([P, NB], BF16)    # DVE: 1/var
        q_ = big.tile([P, NB], BF16)    # DVE h0/h1s1; Pool h1s2/s3: msq+var
        mun2 = big.tile([P, NB], BF16)  # DVE: -2*mu
        m2 = big.tile([P, NB], BF16)    # mun2 * p = -2*mu*p
        muv = big.tile([P, NB], BF16)   # copy of mu
        pq = big.tile([P, NB], BF16)    # p * q
        pc = big.tile([P, HALF], BF16)  # Pool copy of p_ for h1 s2/s3 rhs
        oneh = big.tile([P, RB * NM], F32)    # [128,2048] m-stripe one-hots
        onehb = big.tile([P, RB * NM], BF16)  # bf16 one-hots (mask rhs)

        # Pool-written [128,512] lhsT mirrors (for Pool-written rhs slabs)
        pL = sm.tile([P, RB], BF16)
        qL = sm.tile([P, RB], BF16)
        m2L = sm.tile([P, RB], BF16)
        muvL = sm.tile([P, RB], BF16)

        ioti = sm.tile([P, RB], I32)
        iotP = sm.tile([P, P], I32)
        ones_d = sm.tile([P, P], BF16)  # DVE memset (seed lhsT, DVE rhs)
        ones_p = sm.tile([P, P], BF16)  # Pool memset (seed lhsT, Pool rhs)
        bigIf = sm.tile([P, P], BF16)   # DVE identity
        bigI = sm.tile([P, P], BF16)    # DVE: identity * 1e30
        bias17 = sm.tile([P, 17], F32)
        out38 = sm.tile([P, 38], F32)

        # ---- setup (iotas early; DVE one-hot builds are interleaved
        # into the prep stream below so they stay off the startup path) ----
        nc.gpsimd.iota(ioti, pattern=[[1, RB]], base=0, channel_multiplier=-1)
        nc.gpsimd.iota(iotP, pattern=[[1, P]], base=0, channel_multiplier=-1)
        nc.gpsimd.memset(ones_p, 1.0)
        nc.vector.memset(ones_d, 1.0)

        # ---- input DMAs: sg before mu (recip chain is longest) ----
        for h in range(2):
            for (a, b) in SLABS:
                sl = slice(HALF * h + a, HALF * h + b)
                nc.sync.dma_start(out=sg[:, sl], in_=sgT_d[:, sl])
                nc.sync.dma_start(out=mu[:, sl], in_=muT_d[:, sl])

        # ---- plane prep ----
        lowp = nc.allow_low_precision("bf16 planes feed the PE")
        with lowp:
            # Act: squares for both halves, slab-wise
            for h in range(2):
                for (a, b) in SLABS:
                    sl = slice(HALF * h + a, HALF * h + b)
                    nc.scalar.activation(out=var[:, sl], in_=sg[:, sl],
                                         func=AF.Square)
                    nc.scalar.activation(out=msq[:, sl], in_=mu[:, sl],
                                         func=AF.Square)

            def dve_slab(sl, full):
                nc.vector.reciprocal(p_[:, sl], var[:, sl])
                nc.vector.tensor_scalar_mul(mun2[:, sl], mu[:, sl], -2.0)
                if full:
                    nc.vector.tensor_add(q_[:, sl], msq[:, sl], var[:, sl])
                    nc.vector.tensor_mul(pq[:, sl], p_[:, sl], q_[:, sl])
                    nc.vector.tensor_mul(m2[:, sl], mun2[:, sl], p_[:, sl])
                    nc.vector.tensor_copy(out=muv[:, sl], in_=mu[:, sl])

            # DVE: h0 slabs with the one-hot/mask setup interleaved right
            # after s1 (mask matmul needs onehb at the end of the first jj
            # chain; f32 oneh is for the much later pos extraction), then the
            # h1 p/mun2 feeds (Pool needs p_h1 early)
            dve_slab(slice(0, 512), True)
            for m in range(NM):
                # onehb_m[p, c] = (c - p == 128*m)
                nc.vector.tensor_single_scalar(
                    out=onehb[:, RB * m:RB * (m + 1)], in_=ioti, scalar=P * m,
                    op=ALU.is_equal)
            nc.vector.tensor_single_scalar(out=bigIf, in_=iotP, scalar=0,
                                           op=ALU.is_equal)
            nc.vector.tensor_scalar_mul(bigI, bigIf, BIG)
            dve_slab(slice(512, 1024), True)
            for m in range(NM):
                nc.vector.tensor_single_scalar(
                    out=oneh[:, RB * m:RB * (m + 1)], in_=ioti, scalar=P * m,
                    op=ALU.is_equal)
            dve_slab(slice(1024, 2048), True)
            for (a, b) in SLABS:
                dve_slab(slice(HALF + a, HALF + b), False)

            # Pool: lhsT mirrors (written once h0 s1 planes exist)
            nc.gpsimd.tensor_copy(out=pL, in_=p_[:, 0:RB])
            nc.gpsimd.tensor_copy(out=qL, in_=q_[:, 0:RB])
            nc.gpsimd.tensor_copy(out=m2L, in_=m2[:, 0:RB])
            nc.gpsimd.tensor_copy(out=muvL, in_=muv[:, 0:RB])
            # Pool: h1 rhs planes (all slabs; s1 first so it lands early)
            for (a, b) in SLABS:
                sl = slice(HALF + a, HALF + b)
                cl = slice(a, b)  # pc is [P, 2048] indexed by h1-local col
                nc.gpsimd.tensor_add(q_[:, sl], msq[:, sl], var[:, sl])
                nc.gpsimd.tensor_copy(out=pc[:, cl], in_=p_[:, sl])
                nc.gpsimd.tensor_copy(out=muv[:, sl], in_=mu[:, sl])
                nc.gpsimd.tensor_mul(m2[:, sl], mun2[:, sl], p_[:, sl])
                nc.gpsimd.tensor_mul(pq[:, sl], p_[:, sl], q_[:, sl])

        c16 = 0
        s512_list = {}
        for h in range(2):
            for m in range(NM):
                mblk = slice(P * m, P * (m + 1))
                for th in range(2):
                    t = 2 * h + th  # global column tile: cols TW*t ..
                    u = pp.tile([P, TW], F32, name=f"u{m}{t}", tag="ps")
                    # Orphan bf16 ldweights absorb the PSUM-slot WAR deps (one
                    # wait per matmul in walrus): the exp accum write (Act) and,
                    # for pos tiles, the pos-extract scratch write (DVE).
                    if c16 >= 4:
                        ec = 2 * (17 + c16 - 4)
                        nc.tensor.ldweights(out38.bitcast(BF16)[0:1, ec:ec + 2])
                        if (c16 - 4) in s512_list:
                            nc.tensor.ldweights(
                                s512_list[c16 - 4].bitcast(BF16)[0:1, 0:2])
                    for jj in range(2):
                        g0 = TW * t + RB * jj
                        osl = slice(RB * jj, RB * (jj + 1))
                        gsl = slice(g0, g0 + RB)
                        csl = slice(g0 - HALF, g0 - HALF + RB)  # pc-local
                        has_mask = (t == 0 and jj == 0)
                        pool_side = (h == 1)
                        if pool_side:
                            chunks = [(pL[:, mblk], q_[:, gsl]),
                                      (qL[:, mblk], pc[:, csl]),
                                      (ones_p, pq[:, gsl]),
                                      (m2L[:, mblk], muv[:, gsl]),
                                      (muvL[:, mblk], m2[:, gsl])]
                        else:
                            chunks = [(p_[:, mblk], q_[:, gsl]),
                                      (q_[:, mblk], p_[:, gsl]),
                                      (ones_d, pq[:, gsl]),
                                      (m2[:, mblk], muv[:, gsl]),
                                      (muv[:, mblk], m2[:, gsl])]
                        for ci, (lhsT, rhs) in enumerate(chunks):
                            nc.tensor.matmul(
                                u[:, osl], lhsT=lhsT, rhs=rhs,
                                start=(ci == 0),
                                stop=(ci == len(chunks) - 1 and not has_mask))
                        if has_mask:
                            # diagonal (always in local cols 128m..128m+127):
                            # += 1e30 at (p, 128m+p) so it loses the min and
                            # underflows the exp.
                            nc.tensor.matmul(
                                u[:, osl], lhsT=bigI,
                                rhs=onehb[:, RB * m:RB * (m + 1)],
                                start=False, stop=True)

                    # ---- consumers (read PSUM directly) ----
                    if t == 2:
                        # positive logits live at local cols 128m+p, i.e. the
                        # first 512 of this tile
                        s512 = scr.tile([P, RB], F32, name="s512", tag="s512",
                                        bufs=2)
                        s512_list[c16] = s512
                        nc.vector.tensor_mul(s512, u[:, 0:RB],
                                             oneh[:, RB * m:RB * (m + 1)])
                        nc.vector.tensor_reduce(out38[:, 34 + m:35 + m], s512,
                                                axis=AX.X, op=ALU.add)
                    if c16 == 15:
                        # final tile: consume per 512-col half (two mrow/esum
                        # pairs at cols 15/16), reduces issued before exps so
                        # the drain chain is one 512 reduce + one 512 exp
                        for jj, cc in ((0, 15), (1, 16)):
                            nc.vector.tensor_reduce(
                                out38[:, cc:cc + 1],
                                u[:, RB * jj:RB * (jj + 1)],
                                axis=AX.X, op=ALU.min)
                            nc.vector.tensor_scalar_mul(
                                bias17[:, cc:cc + 1], out38[:, cc:cc + 1], SCL)
                        for jj, cc in ((0, 15), (1, 16)):
                            e2h = scr.tile([P, RB], BF16, name="e2h",
                                           tag="e2h", bufs=2)
                            nc.scalar.activation(
                                out=e2h, in_=u[:, RB * jj:RB * (jj + 1)],
                                func=AF.Exp, bias=bias17[:, cc:cc + 1],
                                scale=-SCL, accum_out=out38[:, 17 + cc:18 + cc])
                    else:
                        if c16 >= 13:
                            # spread the tail: per-jj partial mins overlap the
                            # group's second half
                            pm = scr.tile([P, 2], F32, name="pm", tag="pm",
                                          bufs=2)
                            for jj in range(2):
                                nc.vector.tensor_reduce(
                                    pm[:, jj:jj + 1],
                                    u[:, RB * jj:RB * (jj + 1)],
                                    axis=AX.X, op=ALU.min)
                            nc.vector.tensor_reduce(out38[:, c16:c16 + 1], pm,
                                                    axis=AX.X, op=ALU.min)
                        else:
                            nc.vector.tensor_reduce(out38[:, c16:c16 + 1], u,
                                                    axis=AX.X, op=ALU.min)
                        nc.vector.tensor_scalar_mul(bias17[:, c16:c16 + 1],
                                                    out38[:, c16:c16 + 1], SCL)
                        e2k = scr.tile([P, TW], BF16, name="e2k", tag="e2k",
                                       bufs=2)
                        nc.scalar.activation(
                            out=e2k, in_=u, func=AF.Exp,
                            bias=bias17[:, c16:c16 + 1], scale=-SCL,
                            accum_out=out38[:, 17 + c16:18 + c16])
                    c16 += 1

        nc.sync.dma_start(out=out_d[:, :], in_=out38)

    return nc


_NC_CACHE = None


def _get_nc():
    global _NC_CACHE
    if _NC_CACHE is None:
        nc = _build_nc()
        nc.finalize()  # runs Bacc.compile(): wait legalization for TRN2
        _NC_CACHE = nc
    return _NC_CACHE


def run_sharded(mu_x, sigma_x, mu_p, sigma_p, trace=False):
    import ml_dtypes
    bf16 = ml_dtypes.bfloat16
    mus = np.concatenate([np.asarray(mu_x, np.float32),
                          np.asarray(mu_p, np.float32)], 0)
    sigmas = np.concatenate([np.asarray(sigma_x, np.float32),
                             np.asarray(sigma_p, np.float32)], 0)
    muT = np.ascontiguousarray(mus.T.astype(bf16))
    sgT = np.ascontiguousarray(sigmas.T.astype(bf16))
    in_maps = [
        {"muT": np.ascontiguousarray(np.roll(muT, -RB * k, axis=1)),
         "sigmaT": np.ascontiguousarray(np.roll(sgT, -RB * k, axis=1))}
        for k in range(NC)
    ]
    kwargs = {}
    if trace:
        kwargs = dict(trace=True, trace_cores=[0])
    br = run_bass_kernel_spmd(_get_nc(), in_maps, core_ids=list(range(NC)),
                              **kwargs)
    # host tail in float64: per-row logsumexp over the 4 column tiles' halves
    total = 0.0
    for r in br.results:
        o = r["out"].astype(np.float64)           # [128, 38]
        mrow = o[:, 0:17]                         # per-tile row-min of U'
        esum = o[:, 17:34]
        upos = o[:, 34:38]                        # positive U' per (p, m)
        L = -SCL * mrow + np.log(esum)            # per-tile partial LSEs
        # tile k (k<15): h=k//8, m=(k%8)//2; k in {15,16} -> m=3
        groups = [[c for c in range(15) if (c % 8) // 2 == m] for m in range(NM)]
        groups[3] += [15, 16]
        lse = np.stack([np.logaddexp.reduce(L[:, g], axis=1)
                        for g in groups], axis=1)  # [p, m]
        total += float(np.sum(lse + SCL * upos))
    n_classes = NB - 1
    to_mult = (n_classes - 1.0 / WEIGHT) / (n_classes - 1)
    to_add = -np.log(np.float64(to_mult))
    loss = np.float32(total / NB - to_add)
    return loss, br


def kernel(z_hat, mu_x, sigma_x, mu_p, sigma_p):
    loss, _ = run_sharded(mu_x, sigma_x, mu_p, sigma_p)
    return np.asarray(loss, np.float32)

